# revision 9
# baseline (speedup 1.0000x reference)
"""Trainium2 Bass kernel for the AdaptiveGraphLearner module (V8).

Strategy (data-parallel over batch, 2 batches per core, 8 cores):
  out[i, m] = SRA[i, m] + (blend/2) * dyn2[i, m]
where
  SRA  = (1-blend)/rs_i * relu(static)  (+ diagonal term, host-precomputed)
  dyn2 = row-softmax over the top-32 entries of sim = rep @ rep.T / sqrt(E)
         (softmax restricted to top-k == topk of softmax; the full softmax
          denominator cancels algebraically)

Per [128, 2048] row-block tile on device:
  PE   : sim = rep.T @ rep (fp16 matmuls, K=32) -> PSUM
  ACT  : E = exp(sim * s)  (PSUM -> SBUF f32; f32 keeps the topk
         threshold comparison exact - bf16/fp16 E inflates boundary ties)
  DVE  : 12x max8 chunk candidates, 4x max8 + 3x match_replace ranks
         -> maxb (top-32 values), t32 = maxb[:, 31]
  ACT  : s32/c via activation(Copy, scale=1/c, accum_out=...) over maxb
  DVE/ACT: M = (E >= t32) mask (DVE is_ge {0,1} or ACT Sign {-1,+1};
         split across tiles to balance the two engines)
  PE   : sim += 600 * M            (diag(600) lhsT matmul accumulate)
  ACT  : X' = exp(s*sim' - 600*s)  -> f32 = E*mask (dropped entries
         underflow to 0)
  Pool : X = X' / (s32/c) -> bf16  (gpsimd normalize_recip)
  DMA  : X += SRA row block        (gpsimd SWDGE CCE-add from HBM)
  DMA  : X -> out DRAM (bf16; host upcasts to f32)
"""

import math

import numpy as np

B, N, H, E = 16, 2048, 256, 32
TOPK = 32
NCORES = 8
BPC = B // NCORES          # batches per core
P = 128                    # partitions
NBLK = N // P              # row blocks per batch
MMFREE = 512               # matmul moving free dim
NSEG = N // MMFREE
SCALE = 1.0 / math.sqrt(E)
BIGM = 600.0               # mask offset added to sim for kept entries
SHIFT = BIGM * SCALE       # = 106.066... subtracted via the exp bias

# fraction of tiles whose mask comparison runs on ACT (Sign) vs DVE (is_ge):
# tile does DVE mask when (tile_idx % ACT_M_MOD) == 0
ACT_M_MOD = 4

# top-k candidate extraction config (chunks x top-8)
N_CHUNKS = 10
_base = N // N_CHUNKS
_extra = N - _base * N_CHUNKS
CHUNK_BOUNDS = []
_off = 0
for _c in range(N_CHUNKS):
    _sz = _base + (1 if _c < _extra else 0)
    CHUNK_BOUNDS.append((_off, _off + _sz))
    _off += _sz

_cached = {}


def _build_nc():
    import concourse.bass as bass
    import concourse.bacc as bacc
    import concourse.mybir as mybir
    from concourse.tile import TileContext

    dt = mybir.dt
    f32 = dt.float32
    f16 = dt.float16
    bf16 = dt.bfloat16
    Alu = mybir.AluOpType
    Act = mybir.ActivationFunctionType

    nc = bacc.Bacc(None)

    seqT = nc.declare_dram_parameter("seqT", [BPC, H, N], f16, isOutput=False)
    nbT = nc.declare_dram_parameter("nbT", [BPC, E, N], f16, isOutput=False)
    fpw = nc.declare_dram_parameter("fpw", [H, E], f16, isOutput=False)
    sra = nc.declare_dram_parameter("sra", [N, N], bf16, isOutput=False)
    dg = nc.declare_dram_parameter("dg", [P, P], f16, isOutput=False)
    i32 = nc.declare_dram_parameter("i32", [E, E], f16, isOutput=False)
    cinv = nc.declare_dram_parameter("cinv", [P, 1], f32, isOutput=False)
    out = nc.declare_dram_parameter("out", [BPC, N, N], bf16, isOutput=True)

    with TileContext(nc) as tc:
        with (
            tc.tile_pool(name="persist", bufs=1) as persist,
            tc.tile_pool(name="small", bufs=4) as small,
            tc.tile_pool(name="e_p", bufs=3) as e_p,
            tc.tile_pool(name="m_p", bufs=3) as m_p,
            tc.tile_pool(name="x_p", bufs=3) as x_p,
            tc.tile_pool(name="psum", bufs=2, space="PSUM") as psum_p,
        ):
            # ---- constants ------------------------------------------------
            fpw_t = []
            for k2 in range(2):
                ft = persist.tile([P, E], f16, tag=f"fpw{k2}")
                nc.sync.dma_start(out=ft, in_=fpw[k2 * P:(k2 + 1) * P, :])
                fpw_t.append(ft)
            dg_t = persist.tile([P, P], f16, tag="dg")
            nc.sync.dma_start(out=dg_t, in_=dg[:, :])
            i32_t = persist.tile([E, E], f16, tag="i32")
            nc.sync.dma_start(out=i32_t, in_=i32[:, :])
            cinv_t = persist.tile([P, 1], f32, tag="cinv")
            nc.sync.dma_start(out=cinv_t, in_=cinv[:, :])
            bsh_t = persist.tile([P, 1], f32, tag="bsh")
            nc.vector.memset(bsh_t, -SHIFT)

            # ---- phase A: rep[b] = tanh(fpw.T @ seq + nbT) ----------------
            rep_t = []
            with tc.tile_pool(name="seq_p", bufs=2) as seq_p:
                for b in range(BPC):
                    rt = persist.tile([E, N], f16, tag=f"rep{b}")
                    rep_t.append(rt)
                    ps = psum_p.tile([E, N], f32, tag="sim")
                    sq = []
                    for k2 in range(2):
                        st = seq_p.tile([P, N], f16, tag=f"seq{k2}")
                        nc.sync.dma_start(
                            out=st, in_=seqT[b, k2 * P:(k2 + 1) * P, :]
                        )
                        sq.append(st)
                    nbc = seq_p.tile([E, N], f16, tag="nbc")
                    nc.sync.dma_start(out=nbc, in_=nbT[b, :, :])
                    for j in range(NSEG):
                        lo, hi = j * MMFREE, (j + 1) * MMFREE
                        for k2 in range(2):
                            nc.tensor.matmul(
                                ps[:, lo:hi],
                                lhsT=fpw_t[k2],
                                rhs=sq[k2][:, lo:hi],
                                start=(k2 == 0), stop=False,
                            )
                        # += nbT via identity lhsT (keeps DVE free)
                        nc.tensor.matmul(
                            ps[:, lo:hi],
                            lhsT=i32_t,
                            rhs=nbc[:, lo:hi],
                            start=False, stop=True,
                        )
                    nc.scalar.activation(out=rt, in_=ps, func=Act.Tanh)

            # ---- phase B: 2-stage software pipeline over tiles ------------
            # stage1: sim matmuls, E, candidates, ranks, s32, mask M
            # stage2: 600*M matmul accumulate, X', normalize, CCE add, out
            # Emitting stage1(i+1) before stage2(i) keeps every engine's
            # in-order queue fed with independent work (PE: sim(i+1) is not
            # behind Mmm(i); ACT: E(i+1) is not behind X'(i)).
            def stage1(b, r):
                ps = psum_p.tile([P, N], f32, tag="sim")
                for j in range(NSEG):
                    lo, hi = j * MMFREE, (j + 1) * MMFREE
                    nc.tensor.matmul(
                        ps[:, lo:hi],
                        lhsT=rep_t[b][:, r * P:(r + 1) * P],
                        rhs=rep_t[b][:, lo:hi],
                        start=True, stop=True,
                    )
                e_t = e_p.tile([P, N], f32, tag="e")
                nc.scalar.activation(out=e_t, in_=ps, func=Act.Exp,
                                     scale=SCALE)

                # candidates: top-8 per chunk
                cands = small.tile([P, N_CHUNKS * 8], f32, tag="cands")
                for c, (lo, hi) in enumerate(CHUNK_BOUNDS):
                    nc.vector.max(
                        out=cands[:, c * 8:(c + 1) * 8], in_=e_t[:, lo:hi]
                    )
                # ranks 1..32 of candidates
                maxb = small.tile([P, 32], f32, tag="maxb")
                for rd in range(4):
                    nc.vector.max(out=maxb[:, rd * 8:(rd + 1) * 8],
                                  in_=cands)
                    if rd < 3:
                        nc.vector.match_replace(
                            out=cands,
                            in_to_replace=maxb[:, rd * 8:(rd + 1) * 8],
                            in_values=cands, imm_value=0.0,
                        )

                # s32/c on ACT (accum_out); mask on DVE or ACT
                s32c = small.tile([P, 1], f32, tag="s32c")
                dump = small.tile([P, 32], f32, tag="dump")
                nc.scalar.activation(out=dump, in_=maxb, func=Act.Copy,
                                     scale=cinv_t, accum_out=s32c)
                m_t = m_p.tile([P, N], f16, tag="m")
                if (b * NBLK + r) % ACT_M_MOD == 0:
                    # M = (E >= t32) in {0, 1} on DVE
                    nc.vector.tensor_scalar(
                        out=m_t, in0=e_t, scalar1=maxb[:, 31:32],
                        scalar2=None, op0=Alu.is_ge,
                    )
                else:
                    # M = sign(E - t32*(1-1e-6)) in {-1, +1} on ACT
                    t32n = small.tile([P, 1], f32, tag="t32n")
                    nc.vector.tensor_scalar(
                        out=t32n, in0=maxb[:, 31:32], scalar1=-0.999999,
                        scalar2=None, op0=Alu.mult,
                    )
                    nc.scalar.activation(out=m_t, in_=e_t, func=Act.Sign,
                                         bias=t32n)
                return (b, r, ps, m_t, s32c)

            def stage2(st):
                b, r, ps, m_t, s32c = st
                for j in range(NSEG):
                    lo, hi = j * MMFREE, (j + 1) * MMFREE
                    nc.tensor.matmul(
                        ps[:, lo:hi],
                        lhsT=dg_t,
                        rhs=m_t[:, lo:hi],
                        start=False, stop=True,
                        skip_group_check=True,
                    )
                # X' = E*mask (f32), then X = X'/(s32/c) cast bf16 on
                # gpsimd (normalize_recip); masked entries underflow to 0
                xp_t = x_p.tile([P, N], f32, tag="xp")
                nc.scalar.activation(out=xp_t, in_=ps, func=Act.Exp,
                                     scale=SCALE, bias=bsh_t)
                x_t = x_p.tile([P, N], bf16, tag="x")
                nc.gpsimd.normalize_recip(x_t, xp_t, s32c)
                # X += SRA (CCE add during DMA, no compute engine)
                nc.gpsimd.dma_start(
                    out=x_t, in_=sra[r * P:(r + 1) * P, :],
                    accum_op=Alu.add,
                )
                nc.sync.dma_start(
                    out=out[b, r * P:(r + 1) * P, :], in_=x_t
                )

            tiles = [(b, r) for b in range(BPC) for r in range(NBLK)]
            prev = None
            for (b, r) in tiles:
                cur = stage1(b, r)
                if prev is not None:
                    stage2(prev)
                prev = cur
            stage2(prev)
    nc.finalize()
    return nc


def _prep_inputs(inputs):
    """Host-side sharding + init-time preprocessing. Returns in_maps."""
    import ml_dtypes
    bf16 = ml_dtypes.bfloat16

    seq = np.asarray(inputs["sequence_features"], dtype=np.float32)
    te = np.asarray(inputs["timestep_embedding"], dtype=np.float32)
    sa = np.asarray(inputs["static_adjacency"], dtype=np.float32)
    ne = np.asarray(inputs["node_embeddings"], dtype=np.float32)
    fp_w = np.asarray(inputs["fp_w"], dtype=np.float32)
    fp_b = np.asarray(inputs["fp_b"], dtype=np.float32)
    tp_w = np.asarray(inputs["tp_w"], dtype=np.float32)
    tp_b = np.asarray(inputs["tp_b"], dtype=np.float32)
    blend_logit = float(np.asarray(inputs["blend_logit"]))

    b0 = 1.0 / (1.0 + math.exp(-blend_logit))
    c = b0 / 2.0

    # time conditioning + biases folded into per-batch node embeddings
    tproj = te @ tp_w + tp_b + fp_b                       # [B, E]
    nb = ne[None, :, :] + tproj[:, None, :]               # [B, N, E]
    nbT = np.ascontiguousarray(nb.transpose(0, 2, 1)).astype(np.float16)
    seqT = np.ascontiguousarray(seq.transpose(0, 2, 1)).astype(np.float16)

    # static adjacency: init-time buffer preprocessing
    srelu = np.maximum(sa, 0.0).astype(np.float32)
    rs = (srelu.sum(axis=1, dtype=np.float32) + 1.0).astype(np.float32)
    A = ((1.0 - b0) / rs).astype(np.float32)
    sra_full = (A[:, None] * srelu).astype(np.float32)
    idx = np.arange(N)
    sra_full[idx, idx] += A + np.float32(b0 / 2.0)
    sra_full = sra_full.astype(bf16)

    dg = (np.eye(P, dtype=np.float32) * BIGM).astype(np.float16)
    i32 = np.eye(E, dtype=np.float16)
    cinv = np.full((P, 1), 1.0 / c, dtype=np.float32)

    in_maps = []
    for cc in range(NCORES):
        lo, hi = cc * BPC, (cc + 1) * BPC
        in_maps.append({
            "seqT": seqT[lo:hi],
            "nbT": np.ascontiguousarray(nbT[lo:hi]),
            "fpw": fp_w.astype(np.float16),
            "sra": sra_full,
            "dg": dg,
            "i32": i32,
            "cinv": cinv,
        })
    return in_maps


def kernel(**inputs):
    from concourse.bass_utils import run_bass_kernel_spmd

    if "nc" not in _cached:
        _cached["nc"] = _build_nc()
    nc = _cached["nc"]
    in_maps = _prep_inputs(inputs)
    res = run_bass_kernel_spmd(nc, in_maps, core_ids=list(range(NCORES)))
    out = np.concatenate([res.results[c]["out"] for c in range(NCORES)],
                         axis=0)
    return out.astype(np.float32)
